# revision 4
# baseline (speedup 1.0000x reference)
"""EGCL multi-head message-passing layer on 8 Trainium2 NeuronCores.

Row-parallel decomposition: core c owns node rows i in [96c, 96(c+1)).
Each core computes, for its 96 rows, the O(N^2) pairwise work:
  - phi_e MLP over all (i, j) pairs -> m_ij  [rows, 768, 128]
  - gate e_ij = sigmoid(m_ij @ Winf + binf) (diag-masked)
  - m_i = sum_j e_ij * m_ij
  - phi_x MLP -> px_ij [rows, 768, 4]; equivariant shift
      eq[i,h,d] = sum_j diff[i,j,h,d] * (px+bx3)[i,j,h] / (sqrt(|diff|^2+eps)+1)
The O(N) head (phi_x_cross, phi_h, residuals) runs on host numpy.

Key algebraic restructuring (exact, not approximate): phi_e layer 1 is
  We1.T @ [sqn | hc_j | hc_i] = We1[:4].T@sqn + We1[4:84].T@hc_j + B_i,
so the pairwise matmul has K=4, the hc_j term is a block-constant K=80
matmul, and B_i rides for free as the per-partition bias of the silu.

Layouts are feature-major: tiles are [feature/head, j] with j on the free
dim, so all j-reductions are free-dim accumulations (fused into DVE
scalar_tensor_tensor accum_out) and no transposes are needed anywhere.
"""
import numpy as np

N, H, DH, M = 768, 4, 64, 128
NC = 8                    # cores
NL = N // NC              # 96 rows per core
IB = 8                    # rows per block
NB = NL // IB             # 12 blocks per core
NJ = N                    # free (j) extent
USE_F32R = True

_nc_cache = {}


def _build():
    import concourse.bass as bass  # noqa: F401
    from concourse import bacc
    import concourse.tile as tile
    from concourse import mybir

    F32 = mybir.dt.float32
    F32R = mybir.dt.float32r if USE_F32R else mybir.dt.float32
    AF = mybir.ActivationFunctionType
    ALU = mybir.AluOpType

    nc = bacc.Bacc(None, target_bir_lowering=False)

    def din(name, shape):
        return nc.dram_tensor(name, shape, F32, kind="ExternalInput")

    # weights / static operands
    we1s = din("we1s", [4, M])        # We1 rows 0:4   (sqn part), lhsT
    we1h = din("we1h", [80, M])       # We1 rows 4:84  (hc_j part), lhsT
    we2 = din("we2", [M, M])
    winf = din("winf", [M, 1])
    wx1 = din("wx1", [M, M])
    wx2 = din("wx2", [M, M])
    wx3 = din("wx3", [M, 4])
    hct = din("hct", [80, NJ])        # h_concat.T (feature-major, j on free)
    btl = din("btl", [M, NL])         # B.T columns for this core's rows
    xtd = din("xtd", [3, 32, NJ])     # x[j,h,d] tiled (h,iL)-major per d
    xcols = din("xcols", [3 * NB * 32, 1])  # x_i columns per (d, block)
    hmask = din("hmask", [NB, IB, NJ])      # 0.5 off-diag, 0.0 at local diag
    colc = din("colc", [M, 6])        # packed bias columns, see HOST_COLS
    # col 0: be2, 1: bx1, 2: bx2, 3: bx3 (rows 0:4), 4: binf/2, 5: eps=1e-8

    m_out = nc.dram_tensor("m_out", [M, NL], F32, kind="ExternalOutput")
    eq_out = nc.dram_tensor("eq_out", [NB * 32, 3], F32, kind="ExternalOutput")

    with tile.TileContext(nc) as tc:
        with tc.tile_pool(name="const", bufs=1) as cp, \
             tc.tile_pool(name="geo", bufs=2) as gp, \
             tc.tile_pool(name="geok", bufs=2) as gk, \
             tc.tile_pool(name="mlp", bufs=2) as mp, \
             tc.tile_pool(name="actp", bufs=3) as ap_, \
             tc.tile_pool(name="scr", bufs=1) as scp, \
             tc.tile_pool(name="mpool", bufs=IB + 1) as mmp, \
             tc.tile_pool(name="tail", bufs=2) as tp, \
             tc.tile_pool(name="psA", bufs=3, space="PSUM") as psA, \
             tc.tile_pool(name="psE", bufs=1, space="PSUM") as psE:

            def load(tag, shape, src, dtype=F32R):
                t = cp.tile(shape, dtype, tag=tag)
                ap = src if dtype == F32 else src.bitcast(F32R)
                nc.sync.dma_start(out=t[:], in_=ap)
                return t

            we1s_t = load("we1s", [4, M], we1s[:])
            we1h_t = load("we1h", [80, M], we1h[:])
            we2_t = load("we2", [M, M], we2[:])
            winf_t = load("winf", [M, 1], winf[:])
            wx1_t = load("wx1", [M, M], wx1[:])
            wx2_t = load("wx2", [M, M], wx2[:])
            wx3_t = load("wx3", [M, 4], wx3[:])
            hct_t = load("hct", [80, NJ], hct[:])
            btl_t = load("btl", [M, NL], btl[:], F32)
            colc_t = load("colc", [M, 6], colc[:], F32)
            xtd_t = [load(f"xtd{d}", [32, NJ], xtd[d], F32) for d in range(3)]

            m_iT = cp.tile([M, NL], F32, tag="m_iT")

            CH = [(0, 512), (512, 256)]

            for b in range(NB):
                hm_b = gp.tile([IB, NJ], F32, tag="hm")
                nc.sync.dma_start(out=hm_b[:], in_=hmask[b])
                # ---------------- geometry for the block ----------------
                diff_d, sq_d = [], []
                for d in range(3):
                    xc = gp.tile([32, 1], F32, tag=f"xc{d}")
                    off = (d * NB + b) * 32
                    nc.sync.dma_start(out=xc[:], in_=xcols[off:off + 32, :])
                    dt_ = gk.tile([32, NJ], F32, tag=f"diff{d}")
                    nc.vector.tensor_scalar(out=dt_[:], in0=xtd_t[d][:],
                                            scalar1=xc[:], scalar2=None,
                                            op0=ALU.subtract)
                    sq = gp.tile([32, NJ], F32, tag=f"sq{d}")
                    nc.gpsimd.tensor_mul(sq[:], dt_[:], dt_[:])
                    diff_d.append(dt_)
                    sq_d.append(sq)
                sqa = scp.tile([32, NJ], F32, tag="sqa")
                nc.gpsimd.tensor_add(sqa[:], sq_d[0][:], sq_d[1][:])
                sqn = gk.tile([32, NJ], F32, tag="sqn")
                nc.vector.tensor_add(sqn[:], sqa[:], sq_d[2][:])
                # rn = 1 / (sqrt(sqn + 1e-8) + 1)
                srt = scp.tile([32, NJ], F32, tag="srt")
                nc.scalar.activation(srt[:], sqn[:], AF.Sqrt,
                                     bias=colc_t[0:32, 5:6])
                sp1 = scp.tile([32, NJ], F32, tag="sp1")
                nc.gpsimd.tensor_scalar_add(sp1[:], srt[:], 1.0)
                rn = gk.tile([32, NJ], F32, tag="rn")
                nc.vector.reciprocal_approx_fast(rn[:], sp1[:])

                el_blk = tp.tile([IB, NJ], F32, tag="el_blk")
                G_blk = tp.tile([32, NJ], F32, tag="G_blk")
                m_tiles = []

                # ---------------- per-row MLP chain ----------------
                for il in range(IB):
                    i = b * IB + il
                    sqn_i = mp.tile([4, NJ], F32R, tag="sqn_i")
                    nc.sync.dma_start(out=sqn_i[:],
                                      in_=sqn[il:il + 1 + 3 * IB:IB, :].bitcast(F32R))
                    z1 = psA.tile([M, NJ], F32, tag="big")
                    for lo, sz in CH:
                        nc.tensor.matmul(z1[:, lo:lo + sz], we1s_t[:],
                                         sqn_i[:, lo:lo + sz], start=True, stop=False)
                        nc.tensor.matmul(z1[:, lo:lo + sz], we1h_t[:],
                                         hct_t[:, lo:lo + sz], start=False, stop=True)
                    y1 = ap_.tile([M, NJ], F32R, tag="act")
                    nc.scalar.activation(y1[:], z1[:], AF.Silu,
                                         bias=btl_t[:, i:i + 1])
                    z2 = psA.tile([M, NJ], F32, tag="big")
                    for lo, sz in CH:
                        nc.tensor.matmul(z2[:, lo:lo + sz], we2_t[:],
                                         y1[:, lo:lo + sz], start=True, stop=True)
                    m_t = mmp.tile([M, NJ], F32R, tag="m")
                    nc.scalar.activation(m_t[:], z2[:], AF.Silu,
                                         bias=colc_t[:, 0:1])
                    m_tiles.append(m_t)
                    # gate logit: el = m @ winf; stash 0.5*el + binf/2 in el_blk
                    elp = psE.tile([1, NJ], F32, tag="el")
                    for lo, sz in CH:
                        nc.tensor.matmul(elp[:, lo:lo + sz], winf_t[:],
                                         m_t[:, lo:lo + sz], start=True, stop=True)
                    el_tmp = mp.tile([1, NJ], F32, tag="el_tmp")
                    nc.vector.tensor_scalar(out=el_tmp[:], in0=elp[:],
                                            scalar1=0.5, scalar2=colc_t[0:1, 4:5],
                                            op0=ALU.mult, op1=ALU.add)
                    nc.sync.dma_start(out=el_blk[il:il + 1, :], in_=el_tmp[:])
                    # phi_x trunk
                    t1p = psA.tile([M, NJ], F32, tag="big")
                    for lo, sz in CH:
                        nc.tensor.matmul(t1p[:, lo:lo + sz], wx1_t[:],
                                         m_t[:, lo:lo + sz], start=True, stop=True)
                    t1 = ap_.tile([M, NJ], F32R, tag="act")
                    nc.scalar.activation(t1[:], t1p[:], AF.Silu,
                                         bias=colc_t[:, 1:2])
                    t2p = psA.tile([M, NJ], F32, tag="big")
                    for lo, sz in CH:
                        nc.tensor.matmul(t2p[:, lo:lo + sz], wx2_t[:],
                                         t1[:, lo:lo + sz], start=True, stop=True)
                    t2 = ap_.tile([M, NJ], F32R, tag="act")
                    nc.scalar.activation(t2[:], t2p[:], AF.Silu,
                                         bias=colc_t[:, 2:3])
                    pxp = psA.tile([M, NJ], F32, tag="big")
                    for lo, sz in CH:
                        nc.tensor.matmul(pxp[0:4, lo:lo + sz], wx3_t[:],
                                         t2[:, lo:lo + sz], start=True, stop=True)
                    # G_i = (px + bx3) * rn_i ; scatter into G_blk rows (h*8+il)
                    rn_i = mp.tile([4, NJ], F32, tag="rn_i")
                    nc.sync.dma_start(out=rn_i[:],
                                      in_=rn[il:il + 1 + 3 * IB:IB, :])
                    G_i = mp.tile([4, NJ], F32, tag="G_i")
                    nc.vector.scalar_tensor_tensor(G_i[:], pxp[0:4, :],
                                                   colc_t[0:4, 3:4], rn_i[:],
                                                   ALU.add, ALU.mult)
                    nc.sync.dma_start(out=G_blk[il:il + 1 + 3 * IB:IB, :],
                                      in_=G_i[:])

                # ---------------- block tail ----------------
                e_raw = tp.tile([IB, NJ], F32, tag="e_raw")
                nc.scalar.activation(e_raw[:], el_blk[:], AF.Tanh)
                e_blk = tp.tile([IB, NJ], F32, tag="e_blk")
                # e = (tanh + 1) * halfmask  == sigmoid(el+binf) masked
                nc.vector.scalar_tensor_tensor(e_blk[:], e_raw[:], 1.0,
                                               hm_b[:], ALU.add, ALU.mult)
                for il in range(IB):
                    i = b * IB + il
                    e_row = mp.tile([1, NJ], F32, tag="e_row")
                    nc.sync.dma_start(out=e_row[:], in_=e_blk[il:il + 1, :])
                    ebc = mp.tile([M, NJ], F32, tag="ebc")
                    nc.gpsimd.partition_broadcast(ebc[:], e_row[:])
                    em_scr = scp.tile([M, 1], F32, tag="em_scr")
                    nc.vector.scalar_tensor_tensor(
                        em_scr[:].broadcast_to((M, NJ)),
                        m_tiles[il][:].bitcast(F32), 1.0, ebc[:],
                        ALU.mult, ALU.mult, accum_out=m_iT[:, i:i + 1])
                eq_sb = tp.tile([32, 4], F32, tag="eq_sb")
                for d in range(3):
                    w_scr = scp.tile([32, 1], F32, tag="w_scr")
                    nc.vector.scalar_tensor_tensor(
                        w_scr[:].broadcast_to((32, NJ)), diff_d[d][:], 1.0, G_blk[:],
                        ALU.mult, ALU.mult, accum_out=eq_sb[:, d:d + 1])
                nc.sync.dma_start(out=eq_out[b * 32:(b + 1) * 32, :],
                                  in_=eq_sb[:, 0:3])

            nc.sync.dma_start(out=m_out[:], in_=m_iT[:])
    nc.compile()
    return nc


def _silu(v):
    return v * (1.0 / (1.0 + np.exp(-v)))


def kernel(x, h, We1, be1, We2, be2, Winf, binf,
           Wx1, bx1, Wx2, bx2, Wx3, bx3,
           Wc1, bc1, Wc2, bc2, Wc3, bc3,
           Wh1, bh1, Wh2, bh2, Wh3, bh3):
    from concourse.bass_utils import run_bass_kernel_spmd

    f = np.float32
    x = np.asarray(x, f); h = np.asarray(h, f)
    We1 = np.asarray(We1, f); be1 = np.asarray(be1, f)
    We2 = np.asarray(We2, f); be2 = np.asarray(be2, f)
    Winf = np.asarray(Winf, f); binf = np.asarray(binf, f)
    Wx1 = np.asarray(Wx1, f); bx1 = np.asarray(bx1, f)
    Wx2 = np.asarray(Wx2, f); bx2 = np.asarray(bx2, f)
    Wx3 = np.asarray(Wx3, f); bx3 = np.asarray(bx3, f)
    Wc1 = np.asarray(Wc1, f); bc1 = np.asarray(bc1, f)
    Wc2 = np.asarray(Wc2, f); bc2 = np.asarray(bc2, f)
    Wc3 = np.asarray(Wc3, f); bc3 = np.asarray(bc3, f)
    Wh1 = np.asarray(Wh1, f); bh1 = np.asarray(bh1, f)
    Wh2 = np.asarray(Wh2, f); bh2 = np.asarray(bh2, f)
    Wh3 = np.asarray(Wh3, f); bh3 = np.asarray(bh3, f)

    # ---------------- host-side O(N) prep ----------------
    diffH = x[:, None, :, :] - x[:, :, None, :]          # [n, a, b, 3] = x_b - x_a
    sqh = np.sum(diffH * diffH, axis=-1)                 # [n, H, H]
    hc = np.concatenate([h, sqh.reshape(N, H * H)], axis=1)   # [n, 80]
    B = hc @ We1[84:164] + be1                           # [n, 128]

    xT = x.reshape(N, 12).T.copy()                       # [12(hd), n]
    # xtd[d][h*8+il, j] = x[j, h, d]
    xtd = np.empty((3, 32, NJ), f)
    for d in range(3):
        for hh in range(H):
            xtd[d, hh * IB:(hh + 1) * IB, :] = xT[hh * 3 + d][None, :]

    shared = dict(
        we1s=We1[0:4].copy(), we1h=We1[4:84].copy(), we2=We2,
        winf=Winf, wx1=Wx1, wx2=Wx2, wx3=Wx3, hct=hc.T.copy(), xtd=xtd,
    )
    colc = np.zeros((M, 6), f)
    colc[:, 0] = be2; colc[:, 1] = bx1; colc[:, 2] = bx2
    colc[0:4, 3] = bx3; colc[:, 4] = float(binf[0]) / 2.0; colc[:, 5] = 1e-8
    shared["colc"] = colc

    in_maps = []
    for c in range(NC):
        rows = slice(c * NL, (c + 1) * NL)
        xc_l = x[rows]                                   # [96, 4, 3]
        xcols = np.empty((3, NB, 32), f)
        for d in range(3):
            for b in range(NB):
                for hh in range(H):
                    xcols[d, b, hh * IB:(hh + 1) * IB] = \
                        xc_l[b * IB:(b + 1) * IB, hh, d]
        hm = np.full((NB, IB, NJ), 0.5, f)
        for b in range(NB):
            for il in range(IB):
                hm[b, il, c * NL + b * IB + il] = 0.0
        in_maps.append({**shared,
                        "btl": B[rows].T.copy(),
                        "xcols": xcols.reshape(-1, 1),
                        "hmask": hm})

    key = "egcl"
    if key not in _nc_cache:
        _nc_cache[key] = _build()
    res = run_bass_kernel_spmd(_nc_cache[key], in_maps, list(range(NC)),
                               **_run_kwargs)
    return _postprocess(res.results, x, h, sqh, diffH,
                        Wc1, bc1, Wc2, bc2, Wc3, bc3,
                        Wh1, bh1, Wh2, bh2, Wh3, bh3)


_run_kwargs = {}


def _postprocess(results, x, h, sqh, diffH,
                 Wc1, bc1, Wc2, bc2, Wc3, bc3,
                 Wh1, bh1, Wh2, bh2, Wh3, bh3):
    f = np.float32
    # ---------------- gather + host tail ----------------
    m_i = np.empty((N, M), f)
    eq = np.empty((N, H, 3), f)
    for c in range(NC):
        m_i[c * NL:(c + 1) * NL] = results[c]["m_out"].T
        eqs = results[c]["eq_out"]                       # [NB*32, 3]
        for b in range(NB):
            blk = eqs[b * 32:(b + 1) * 32]               # rows h*8+il
            eq[c * NL + b * IB:c * NL + (b + 1) * IB] = \
                blk.reshape(H, IB, 3).transpose(1, 0, 2)

    # phi_x_cross on host
    c1 = _silu(m_i @ Wc1 + bc1)
    c2 = _silu(c1 @ Wc2 + bc2)
    pc = (c2 @ Wc3 + bc3).reshape(N, H, H)
    norm_h = np.sqrt(sqh + 1e-8) + 1.0
    ndh = diffH / norm_h[..., None]
    shift_cross = np.einsum("nijd,nij->njd", ndh, pc).astype(f)

    x_new = x + (eq + shift_cross) / f(N - 1)

    # phi_h on host
    ph_in = np.concatenate([m_i, h], axis=1)
    mid = _silu(ph_in @ Wh1 + bh1) @ Wh2 + bh2
    h_new = h + (mid @ Wh3 + bh3)

    return x_new.astype(f), h_new.astype(f)
